# revision 25
# baseline (speedup 1.0000x reference)
"""Multi-head attention (B=8, S=1024, D=1024, H=16, dh=64) on 8 trn2 cores.

Sharding: data-parallel over batch — one batch element per NeuronCore, no
collectives. Per core the kernel computes, in fp16 with fp32 accumulation:

  K^T = Wk^T X^T, Q^T = Wq^T X^T  ([F on partitions, S free]; a head pair
                                   occupies partitions 0:64 / 64:128)
  V   = X Wv      [S part, F free] with a ones column per head (denominator)
  per head h:
    S^T = K_h Q_h^T        ([Sk part, Sq free]; K=64 contraction — the two
                            heads of a pair sit in different PE row groups
                            and their matmuls run concurrently)
    E^T = exp(S^T / 8)     (ScalarE, fused scale, fp16 out)
    [O | d] = E^T.T [V_h|1]  (PSUM [Sq, 65]; col 64 = softmax denominator)
    out[:, h] = O * (1/d)  (VectorE reciprocal + per-partition scale)

The kernel is emitted as one software pipeline over head pairs —
[K-proj(m), Q-proj(m), scores+exp(m), AV(m-1)] — with the V projection
woven between the first two pairs, so ScalarE (exp is the longest single
engine stream after PE) starts ~20us in and never waits long. Output is
written per (pair, row-tile) so store DMAs overlap compute.

Host side only reshapes: slices the batch, transposes X to X^T and casts
fp32->fp16 (the rounding the on-chip matmuls would apply anyway).
"""

import numpy as np

S = 1024   # sequence length (queries == keys)
D = 1024   # model dim
F = 1024   # heads * head_dim
H = 16
DH = 64
P = 128
NCORES = 8
C = 68     # per-head column stride in the V buffer (64 vals + 1 ones + pad)

_cached_nc = None


def _build_nc():
    import concourse.tile as tile
    from concourse import bacc, mybir

    f32 = mybir.dt.float32
    f16 = mybir.dt.float16
    Exp = mybir.ActivationFunctionType.Exp

    nc = bacc.Bacc("TRN2", target_bir_lowering=False, debug=False,
                   num_devices=NCORES)

    xq_t = nc.dram_tensor("xq_t", [D, S], f16, kind="ExternalInput").ap()
    xk_t = nc.dram_tensor("xk_t", [D, S], f16, kind="ExternalInput").ap()
    xv_t = nc.dram_tensor("xv_t", [D, S], f16, kind="ExternalInput").ap()
    wq = nc.dram_tensor("wq", [D, F], f16, kind="ExternalInput").ap()
    wk = nc.dram_tensor("wk", [D, F], f16, kind="ExternalInput").ap()
    wv = nc.dram_tensor("wv", [D, F], f16, kind="ExternalInput").ap()
    out = nc.dram_tensor("out", [S, F], f32, kind="ExternalOutput").ap()

    KD = D // P   # 8 contraction tiles

    with tile.TileContext(nc) as tc:
        with (
            tc.tile_pool(name="persist", bufs=1) as persist,
            tc.tile_pool(name="inputs", bufs=1) as inputs,
            tc.tile_pool(name="e_pool", bufs=6) as e_pool,
            tc.tile_pool(name="kq_ring", bufs=2) as kq_ring,
            tc.tile_pool(name="pout", bufs=2) as pout,
            tc.tile_pool(name="small", bufs=4) as small,
            tc.tile_pool(name="pp_ps", bufs=2, space="PSUM") as pp_ps,
            tc.tile_pool(name="s_ps", bufs=2, space="PSUM") as s_ps,
            tc.tile_pool(name="o_ps", bufs=2, space="PSUM") as o_ps,
        ):
            v65 = persist.tile([P, S // P, H * C], f16, tag="v65")
            v_heads = v65.rearrange("p s (h c) -> p s h c", c=C)
            nc.gpsimd.memset(v_heads[:, :, :, DH:DH + 1], 1.0)

            def load(dram, tag):
                t = inputs.tile([P, KD, 1024], f16, tag=tag)
                for dc in range(KD):
                    nc.sync.dma_start(t[:, dc, :], dram[dc * P:(dc + 1) * P, :])
                return t

            def load2(dram_x, dram_w, tag):
                tx = inputs.tile([P, KD, 1024], f16, tag=tag + "x")
                tw = inputs.tile([P, KD, 1024], f16, tag=tag + "w")
                for dc in range(KD):
                    nc.sync.dma_start(tx[:, dc, :],
                                      dram_x[dc * P:(dc + 1) * P, :])
                    nc.sync.dma_start(tw[:, dc, :],
                                      dram_w[dc * P:(dc + 1) * P, :])
                return tx, tw

            xk_sb, wk_sb = load2(xk_t, wk, "k")
            xq_sb, wq_sb = load2(xq_t, wq, "q")
            # xv/wv live in the E pool: their slots recycle into E tiles
            # once the V projection has consumed them
            xv_sb = e_pool.tile([P, KD, 1024], f16, tag="e", name="xv")
            wv_sb = e_pool.tile([P, KD, 1024], f16, tag="e", name="wv")
            for dc in range(KD):
                nc.sync.dma_start(xv_sb[:, dc, :], xv_t[dc * P:(dc + 1) * P, :])
                nc.sync.dma_start(wv_sb[:, dc, :], wv[dc * P:(dc + 1) * P, :])

            def proj_m(psum_pool, ptag, lhs_sb, rhs_sb, m):
                pss = [psum_pool.tile([P, 512], f32, tag=ptag,
                                      name=f"pj{j}") for j in range(2)]
                for j in range(2):
                    for dc in range(KD):
                        nc.tensor.matmul(
                            pss[j][:, :],
                            lhsT=lhs_sb[:, dc, m * P:(m + 1) * P],
                            rhs=rhs_sb[:, dc, j * 512:(j + 1) * 512],
                            start=(dc == 0), stop=(dc == KD - 1),
                        )
                return pss

            def kq_proj(m):
                kc = kq_ring.tile([P, S], f16, tag="kc")
                qc = kq_ring.tile([P, S], f16, tag="qc")
                pss = proj_m(pp_ps, "pp", wk_sb, xk_sb, m)
                for j in range(2):
                    nc.vector.tensor_copy(kc[:, j * 512:(j + 1) * 512],
                                          pss[j][:, :])
                pss = proj_m(pp_ps, "pp", wq_sb, xq_sb, m)
                for j in range(2):
                    nc.vector.tensor_copy(qc[:, j * 512:(j + 1) * 512],
                                          pss[j][:, :])
                return kc, qc

            def v_proj(m):
                pss = proj_m(o_ps, "o", xv_sb, wv_sb, m)
                for j in range(2):
                    src = pss[j].rearrange("p (h c) -> p h c", c=DH)
                    dst = v_heads[:, m, j * 8:(j + 1) * 8, 0:DH]
                    nc.vector.tensor_copy(dst, src)

            def scores_exp(kc, qc, heads=(0, 1)):
                es = {}
                for i in heads:
                    es[i] = e_pool.tile([P, S // P, S], f16, tag="e",
                                        name=f"e{i}")
                for skm in range(8):
                    pss = {i: s_ps.tile([P, S], f32, tag="s", name=f"s{i}")
                           for i in heads}
                    for j in range(2):
                        for i in heads:
                            b0 = i * DH
                            nc.tensor.matmul(
                                pss[i][:, j * 512:(j + 1) * 512],
                                lhsT=kc[b0:b0 + DH, skm * P:(skm + 1) * P],
                                rhs=qc[b0:b0 + DH, j * 512:(j + 1) * 512],
                                start=True, stop=True,
                            )
                    for i in heads:
                        nc.scalar.activation(es[i][:, skm, :], pss[i][:, :],
                                             Exp, scale=0.125)
                return es

            def av_norm(hp, es, heads=(0, 1), po=None):
                if po is None:
                    po = pout.tile([P, S // P, P], f32, tag="po",
                                   name=f"po{min(heads)}")
                chains = [(sqm, i) for sqm in range(8) for i in heads]
                for qs in range(0, len(chains), 4):
                    quad = chains[qs:qs + 4]
                    nq = len(quad)
                    # 4 chains share one PSUM bank: the first matmul's
                    # start=True clears the whole bank's has_written bits, so
                    # each later chain's first matmul overwrites fresh
                    ps_o = o_ps.tile([P, 512], f32, tag="o")
                    for c, (sqm, i) in enumerate(quad):
                        h = 2 * hp + i
                        for kt in range(8):
                            nc.tensor.matmul(
                                ps_o[:, c * P:c * P + DH + 1],
                                lhsT=es[i][:, kt, sqm * P:(sqm + 1) * P],
                                rhs=v65[:, kt, h * C:h * C + DH + 1],
                                start=(c == 0 and kt == 0),
                                stop=(c == nq - 1 and kt == 7),
                                skip_group_check=True,
                            )
                    st = small.tile([P, 4, DH + 1], f32, tag="st")
                    src_v = ps_o.rearrange("p (c x) -> p c x", x=P)
                    nc.vector.tensor_copy(st[:, 0:nq, :],
                                          src_v[:, 0:nq, 0:DH + 1])
                    rt = small.tile([P, 4, 1], f32, tag="r")
                    nc.vector.reciprocal(rt[:, 0:nq, :],
                                         st[:, 0:nq, DH:DH + 1])
                    done = set()
                    for c, (sqm, i) in enumerate(quad):
                        nc.vector.tensor_scalar_mul(
                            po[:, sqm, i * DH:(i + 1) * DH],
                            st[:, c, 0:DH], rt[:, c, :])
                        done.add(sqm)
                    if max(heads) == 1:
                        for sqm in sorted(done):
                            nc.sync.dma_start(
                                out[sqm * P:(sqm + 1) * P,
                                    hp * P:(hp + 1) * P],
                                po[:, sqm, :])

            # software pipeline over head pairs
            e_prev = None
            LAST = H // 2 - 1
            for hp in range(LAST):
                kc, qc = kq_proj(hp)
                es = scores_exp(kc, qc)
                if hp == 0:
                    for m in range(4):
                        v_proj(m)
                elif hp == 1:
                    for m in range(4, 8):
                        v_proj(m)
                if e_prev is not None:
                    av_norm(hp - 1, e_prev)
                e_prev = es
            # last pair head-granular: head 14's AV overlaps head 15's exp
            kc, qc = kq_proj(LAST)
            es_a = scores_exp(kc, qc, heads=(0,))
            av_norm(LAST - 1, e_prev)
            es_b = scores_exp(kc, qc, heads=(1,))
            po_last = pout.tile([P, S // P, P], f32, tag="po", name="polast")
            av_norm(LAST, es_a, heads=(0,), po=po_last)
            av_norm(LAST, es_b, heads=(1,), po=po_last)

    nc.compile()
    return nc


def _get_nc():
    global _cached_nc
    if _cached_nc is None:
        _cached_nc = _build_nc()
    return _cached_nc


def _in_maps(queries, keys, values, Wq, Wk, Wv):
    f16 = np.float16
    wqb = np.ascontiguousarray(Wq).astype(f16)
    wkb = np.ascontiguousarray(Wk).astype(f16)
    wvb = np.ascontiguousarray(Wv).astype(f16)
    maps = []
    for b in range(NCORES):
        maps.append({
            "xq_t": queries[b].T.astype(f16),
            "xk_t": keys[b].T.astype(f16),
            "xv_t": values[b].T.astype(f16),
            "wq": wqb, "wk": wkb, "wv": wvb,
        })
    return maps


def kernel(queries, keys, values, Wq, Wk, Wv, _trace=False):
    from concourse import bass_utils

    queries = np.asarray(queries)
    keys = np.asarray(keys)
    values = np.asarray(values)
    Wq, Wk, Wv = np.asarray(Wq), np.asarray(Wk), np.asarray(Wv)
    nc = _get_nc()
    maps = _in_maps(queries, keys, values, Wq, Wk, Wv)
    res = bass_utils.run_bass_kernel_spmd(
        nc, maps, core_ids=list(range(NCORES)), trace=_trace)
    out = np.stack([res.results[b]["out"] for b in range(NCORES)])
    if _trace:
        kernel.last_results = res
    return out
